# revision 10
# baseline (speedup 1.0000x reference)
"""Trainium2 Bass kernel for nn_DetectionHead (NMS detection head).

Computes, for x[8, 2048, 2048] f32:
    xp  = relu(x - eps)
    xm  = 3x3 hole-excluded neighborhood max of xp (zero padding)
    out = xp * (x > xm)

Sharding: batch (8 images) across the 8 NeuronCores, data parallel.
The host pads each image with a 1-pixel zero border ([2050, 2050]) so the
device kernel needs no boundary special-casing (pad 0 <= eps behaves exactly
like the reference's zero-padded relu pool).

Per-core layout: full-width row bands. Band t covers image rows
[512t, 512t+512); partition p holds padded rows 512t+4p .. 512t+4p+5 (4 data
rows + 2 halo rows) x the full padded width, so the whole 3x3 stencil is
free-dim-local (compute-engine APs must start at partition 0/32/64/96 on
TRN2, so partition-shifted operands are not an option), and every DMA chunk
is a full ~8.2KB row.  The pipeline per band is FIVE DVE ops (the fp32
tensor_tensor path runs at 1 elem/cycle/lane, so op count == cycles/elem):

    v  = max(row-above, row-below)            TT max  (vertical hole pair)
    c  = max(v, center)                       TT max  (3-tall column max)
    m1 = max(c@col-1, c@col+1)                TT max  (in-place onto c)
    q  = max(m1, v, 2*eps)                    STT (max,max), in-place
    out = select(q - eps < x, x - eps, 0)     custom DVE op (NMS_TAIL_ANT),
                                              writes float16 directly

All five ops stream FLAT contiguous per-partition APs (rows x padded width
coalesced into one dim); the 2 junk columns per row that shifted taps
produce land in the tiles' pad columns and are never stored.  The custom op
replaces the baseline's two-op tail (is_lt compare + mult) and casts to f16
on the write port, halving store traffic.  Bands 0 and 3 run as two
half-width chains so the first load and last store are mostly hidden.

GPSIMD cannot help: the NeuronCore-v3 ISA rejects TensorTensor on the Pool
engine (verified: walrus neuron_isa_check_opcode_on_engine fails), and
16-bit pooling is excluded by accuracy (measured rel_err ~9e-2 vs the 2e-2
budget; the local-max compare needs ~17 significant bits).

Exactness vs the reference (fp32):
 - relu is monotone, so max_i relu(x_i - eps) == relu(max_i x_i - eps); the
   pool runs on raw x.
 - x > relu(m - eps) == (x > m - eps) & (x > 0); with the out-factor
   relu(x - eps) the (x > 0) term can be strengthened to (x > eps).
 - max(m - eps, eps) == max(m, 2*eps) - eps holds exactly in fp32, so
   select(q - eps < x, ...) == (x > m - eps) & (x > eps) exactly.
 - when the mask is 1, x > eps so (x - eps) == relu(x - eps); when 0 the
   output is exactly 0.  The only inexactness is the final f16 store
   (measured rel_err 2.1e-4, gate is 2e-2).
"""

import numpy as np

import concourse.bacc as bacc
import concourse.mybir as mybir
import concourse.tile as tile
from concourse import bass_utils
from concourse.ap import AP

EPS = 0.01
EPS2 = float(np.float32(0.01) * 2)  # exact 2*fl(eps)
B, H, W = 8, 2048, 2048
HP2, WP2 = H + 2, W + 2   # host-padded image
P = 128                   # SBUF partitions
RB = 4                    # rows per partition per band
BAND_H = RB * P           # 512 image rows per band
NBAND = H // BAND_H       # 4 bands
SB = RB + 2               # row slots incl halo
HALF = W // 2             # 1024
F32 = mybir.dt.float32
F16 = mybir.dt.float16
MX = mybir.AluOpType.max


def _register_nms_tail():
    """Register the fused NMS tail as a custom DVE op (per-NEFF uop table;
    no firmware change).  out = select(in1 - s0 < in0, in0 - s0, 0)."""
    from concourse import dve_ops
    from concourse.dve_spec import Spec, Src0, Src1, C0, Zero, select, lower
    from concourse.dve_spec import _has_src1 as has_src1
    from concourse.dve_uop import DveOpSpec
    from concourse.dve_table_gen import dve_ver_for

    name = "NMS_TAIL_ANT"
    for op in dve_ops.OPS:
        if op.name == name:
            return op
    spec = Spec(
        body=select(Src1 - C0 < Src0, Src0 - C0, Zero),
        reference=lambda in0, in1, s0, s1, imm2: np.where(
            (in1 - np.float32(s0)) < in0,
            (in0 - np.float32(s0)).astype(np.float32),
            np.float32(0.0),
        ).astype(np.float32),
    )
    op = dve_ops.DveOp(name, spec, subdim=False, uops_sha={})
    dve_ops.OPS.append(op)
    row = dve_ops._CUSTOM_DVE_ROW_BASE + len(dve_ops.OPS) - 1
    dve_ops._SUB_OPCODE_FOR_NAME[name] = row
    dve_ops.CUSTOM_DVE_SPECS[name] = spec
    for ver in {dve_ver_for("TRN2"), dve_ver_for("TRN3")}:
        compiled = DveOpSpec(name=name, opcode=row, uops=lower(spec, ver=ver),
                             rd1_en=has_src1(spec))
        op.uops_sha[ver] = compiled.sha(ver)
    return op


NMS_TAIL = _register_nms_tail()


def _flat(t, start, length):
    """Flat per-partition view of a tile: [P, length] at free-elem offset
    `start` (tiles are row-major in the free dims, so rows coalesce)."""
    a = t[:]
    return AP(a.tensor, a.offset + start, [list(a.ap[0]), [1, length]])


def _chain(nc, xt, v, c, o, w2):
    """The 5-op DVE chain over a [P, SB, w2] input tile.  All APs are flat
    [P, RB*w2] streams; the shifted taps write junk into each row's 2 pad
    columns, which the store skips.  v/c/o are [P, RB, w2] tiles."""
    L = RB * w2

    nc.vector.tensor_tensor(
        out=_flat(v, 0, L), in0=_flat(xt, 0, L), in1=_flat(xt, 2 * w2, L),
        op=MX,
    )
    nc.vector.tensor_tensor(
        out=_flat(c, 0, L), in0=_flat(v, 0, L), in1=_flat(xt, w2, L), op=MX
    )
    # m1 = max(c@col-1, c@col+1), in place (write trails both reads)
    nc.vector.tensor_tensor(
        out=_flat(c, 0, L - 2), in0=_flat(c, 0, L - 2), in1=_flat(c, 2, L - 2),
        op=MX,
    )
    # q = max(m1, v, 2*eps): the 2eps clamp makes the tail's compare imply
    # x > eps, so no final relu is needed.
    nc.vector.scalar_tensor_tensor(
        out=_flat(c, 0, L - 1), in0=_flat(c, 0, L - 1), scalar=EPS2,
        in1=_flat(v, 1, L - 1), op0=MX, op1=MX,
    )
    # out = select(q - eps < x, x - eps, 0), f16 on the write port
    nc.vector._custom_dve(
        NMS_TAIL, out=_flat(o, 0, L), in0=_flat(xt, w2 + 1, L),
        in1=_flat(c, 0, L), s0=EPS,
    )


def _emit_pipeline(nc, tc, x_d, o_d, out_row_stride, out_offset0, mode="full"):
    """Row-band pipeline; bands 0 and NBAND-1 are column-split into two
    half-width chains (hides the first load / last store)."""
    do_load = mode in ("full", "dmaonly", "loadonly")
    do_store = mode in ("full", "dmaonly", "storeonly")
    do_compute = mode in ("full", "nodma")

    def load(dst, t, cb, w2, strips=2):
        """Column-strip the load across `strips` dma queues so the transfer
        runs at strips x the single-queue rate (the consumer waits for all
        strips; only the critical first load needs strips=4)."""
        if do_load:
            bounds = [w2 * i // strips for i in range(strips + 1)]
            for s0, s1 in zip(bounds, bounds[1:]):
                nc.sync.dma_start(
                    out=dst[:, :, s0:s1],
                    in_=AP(
                        x_d.tensor,
                        t * BAND_H * WP2 + cb + s0,
                        [[RB * WP2, P], [WP2, SB], [1, s1 - s0]],
                    ),
                )
        # nodma: leave the tile unwritten (uninitialized reads are fine for
        # the timing diagnostic; adding memsets would bottleneck the bench)

    def store(src, t, cb, w, strips=2):
        if do_store:
            bounds = [w * i // strips for i in range(strips + 1)]
            for s0, s1 in zip(bounds, bounds[1:]):
                nc.sync.dma_start(
                    out=AP(
                        o_d.tensor,
                        out_offset0 + t * BAND_H * out_row_stride + cb + s0,
                        [[RB * out_row_stride, P], [out_row_stride, RB],
                         [1, s1 - s0]],
                    ),
                    in_=src[:, :, s0:s1],
                )

    with (
        tc.tile_pool(name="iox", bufs=2) as iox,
        tc.tile_pool(name="work", bufs=1) as wp,
        tc.tile_pool(name="ioo", bufs=2) as ioo,
    ):
        for t in range(NBAND):
            split = t in (0, NBAND - 1)
            if split and do_compute:
                for cb in (0, HALF):
                    first = t == 0 and cb == 0
                    last = t == NBAND - 1 and cb == HALF
                    xth = iox.tile([P, SB, HALF + 2], F32, tag="xt")
                    vh = wp.tile([P, RB, HALF + 2], F32, tag="v")
                    ch = wp.tile([P, RB, HALF + 2], F32, tag="c")
                    oh = ioo.tile([P, RB, HALF + 2], F16, tag="o")
                    load(xth, t, cb, HALF + 2, strips=8 if first else 4)
                    _chain(nc, xth, vh, ch, oh, HALF + 2)
                    store(oh, t, cb, HALF, strips=4 if last else 2)
                continue

            xt = iox.tile([P, SB, WP2], F32, tag="xt")
            o = ioo.tile([P, RB, WP2], F16, tag="o")
            load(xt, t, 0, WP2, strips=4)
            if do_compute:
                v = wp.tile([P, RB, WP2], F32, tag="v")
                c = wp.tile([P, RB, WP2], F32, tag="c")
                _chain(nc, xt, v, c, o, WP2)
            store(o, t, 0, W)


def _build_program():
    nc = bacc.Bacc(
        "TRN2",
        target_bir_lowering=False,
        debug=False,
        enable_asserts=False,
        num_devices=B,
    )
    x_d = nc.dram_tensor("x", [HP2, WP2], F32, kind="ExternalInput").ap()
    o_d = nc.dram_tensor("out", [H, W], F16, kind="ExternalOutput").ap()
    with tile.TileContext(nc) as tc:
        _emit_pipeline(nc, tc, x_d, o_d, W, 0)
    nc.compile()
    return nc


def _build_timing_program(niter=1, mode="full"):
    """Same pipeline repeated `niter` times by a device-side loop, writing
    out as f16 [HP2, WP2].  One execute performs niter full passes, so
    (wall(niter) - wall(1)) / (niter - 1) isolates device time from the
    (identical) transfer cost.  Border cells of out are never written."""
    nc = bacc.Bacc(
        "TRN2",
        target_bir_lowering=False,
        debug=False,
        enable_asserts=False,
        num_devices=B,
    )
    di = nc.dram_tensor("x", [1, 8], F32, kind="ExternalInput").ap()
    do = nc.dram_tensor("out", [1, 8], F32, kind="ExternalOutput").ap()
    # the working image lives in Internal DRAM scratch (contents irrelevant
    # for timing); external I/O is a tiny dummy so transfers are ~free.
    x_d = nc.dram_tensor("xi", [HP2, WP2], F32, kind="Internal").ap()
    o_d = nc.dram_tensor("oi", [HP2, WP2], F16, kind="Internal").ap()
    with tile.TileContext(nc) as tc:
        with tc.tile_pool(name="dummy", bufs=1) as dp:
            dt = dp.tile([1, 8], F32, tag="dummy")
            nc.sync.dma_start(out=dt[:], in_=di[:])
            nc.sync.dma_start(out=do[:], in_=dt[:])
        if niter == 1:
            _emit_pipeline(nc, tc, x_d, o_d, WP2, WP2 + 1, mode)
        else:
            with tc.For_i(0, niter, 1):
                _emit_pipeline(nc, tc, x_d, o_d, WP2, WP2 + 1, mode)
    nc.compile()
    return nc


_NC = None


def _get_program():
    global _NC
    if _NC is None:
        _NC = _build_program()
    return _NC


def kernel(x: np.ndarray) -> np.ndarray:
    x = np.asarray(x, dtype=np.float32)
    assert x.shape == (B, H, W), x.shape
    xpad = np.zeros((B, HP2, WP2), dtype=np.float32)
    xpad[:, 1 : H + 1, 1 : W + 1] = x
    nc = _get_program()
    in_maps = [{"x": xpad[i]} for i in range(B)]
    res = bass_utils.run_bass_kernel_spmd(nc, in_maps, core_ids=list(range(B)))
    return np.stack([r["out"] for r in res.results], axis=0).astype(np.float32)


# revision 14
# speedup vs baseline: 1.2342x; 1.2342x over previous
"""Trainium2 Bass kernel for nn_DetectionHead (NMS detection head).

Computes, for x[8, 2048, 2048] f32:
    xp  = relu(x - eps)
    xm  = 3x3 hole-excluded neighborhood max of xp (zero padding)
    out = xp * (x > xm)

Sharding: batch (8 images) across the 8 NeuronCores, data parallel.
The host pads each image with a 1-pixel zero border ([2050, 2050]) so the
device kernel needs no boundary special-casing (pad 0 <= eps behaves exactly
like the reference's zero-padded relu pool).

Per-core layout: full-width row bands. Band t covers image rows
[512t, 512t+512); partition p holds padded rows 512t+4p .. 512t+4p+5 (4 data
rows + 2 halo rows) x the full padded width, so the whole 3x3 stencil is
free-dim-local (compute-engine APs must start at partition 0/32/64/96 on
TRN2, so partition-shifted operands are not an option), and every DMA chunk
is a full ~8.2KB row.  The pipeline per band is FIVE DVE ops (the fp32
tensor_tensor path runs at 1 elem/cycle/lane, so op count == cycles/elem):

    v  = max(row-above, row-below)            TT max  (vertical hole pair)
    c  = max(v, center)                       TT max  (3-tall column max)
    m1 = max(c@col-1, c@col+1)                TT max  (in-place onto c)
    q  = max(m1, v, 2*eps)                    STT (max,max), in-place
    out = select(q - eps < x, x - eps, 0)     custom DVE op (NMS_TAIL_ANT),
                                              writes float16 directly

All five ops stream FLAT contiguous per-partition APs (rows x padded width
coalesced into one dim); the 2 junk columns per row that shifted taps
produce land in the tiles' pad columns and are never stored.  The custom op
replaces the baseline's two-op tail (is_lt compare + mult) and casts to f16
on the write port, halving store traffic.  Bands 0 and 3 run as two
half-width chains so the first load and last store are mostly hidden.

GPSIMD cannot help: the NeuronCore-v3 ISA rejects TensorTensor on the Pool
engine (verified: walrus neuron_isa_check_opcode_on_engine fails), and
16-bit pooling is excluded by accuracy (measured rel_err ~9e-2 vs the 2e-2
budget; the local-max compare needs ~17 significant bits).

Exactness vs the reference (fp32):
 - relu is monotone, so max_i relu(x_i - eps) == relu(max_i x_i - eps); the
   pool runs on raw x.
 - x > relu(m - eps) == (x > m - eps) & (x > 0); with the out-factor
   relu(x - eps) the (x > 0) term can be strengthened to (x > eps).
 - max(m - eps, eps) == max(m, 2*eps) - eps holds exactly in fp32, so
   select(q - eps < x, ...) == (x > m - eps) & (x > eps) exactly.
 - when the mask is 1, x > eps so (x - eps) == relu(x - eps); when 0 the
   output is exactly 0.  The only inexactness is the final f16 store
   (measured rel_err 2.1e-4, gate is 2e-2).
"""

import numpy as np

import concourse.bacc as bacc
import concourse.mybir as mybir
import concourse.tile as tile
from concourse import bass_utils
from concourse.ap import AP

EPS = 0.01
EPS2 = float(np.float32(0.01) * 2)  # exact 2*fl(eps)
B, H, W = 8, 2048, 2048
HP2, WP2 = H + 2, W + 2   # host-padded image
P = 128                   # SBUF partitions
RB = 4                    # rows per partition per band
BAND_H = RB * P           # 512 image rows per band
NBAND = H // BAND_H       # 4 bands
SB = RB + 2               # row slots incl halo
HALF = W // 2             # 1024
F32 = mybir.dt.float32
F16 = mybir.dt.float16
MX = mybir.AluOpType.max


def _register_nms_tail():
    """Register the fused NMS tail as a custom DVE op (per-NEFF uop table;
    no firmware change).  out = select(in1 - s0 < in0, in0 - s0, 0)."""
    from concourse import dve_ops
    from concourse.dve_spec import Spec, Src0, Src1, C0, Zero, select, lower
    from concourse.dve_spec import _has_src1 as has_src1
    from concourse.dve_uop import DveOpSpec
    from concourse.dve_table_gen import dve_ver_for

    name = "NMS_TAIL_ANT"
    for op in dve_ops.OPS:
        if op.name == name:
            return op
    spec = Spec(
        body=select(Src1 - C0 < Src0, Src0 - C0, Zero),
        reference=lambda in0, in1, s0, s1, imm2: np.where(
            (in1 - np.float32(s0)) < in0,
            (in0 - np.float32(s0)).astype(np.float32),
            np.float32(0.0),
        ).astype(np.float32),
    )
    op = dve_ops.DveOp(name, spec, subdim=False, uops_sha={})
    dve_ops.OPS.append(op)
    row = dve_ops._CUSTOM_DVE_ROW_BASE + len(dve_ops.OPS) - 1
    dve_ops._SUB_OPCODE_FOR_NAME[name] = row
    dve_ops.CUSTOM_DVE_SPECS[name] = spec
    for ver in {dve_ver_for("TRN2"), dve_ver_for("TRN3")}:
        compiled = DveOpSpec(name=name, opcode=row, uops=lower(spec, ver=ver),
                             rd1_en=has_src1(spec))
        op.uops_sha[ver] = compiled.sha(ver)
    return op


NMS_TAIL = _register_nms_tail()


def _flat(t, start, length):
    """Flat per-partition view of a tile: [P, length] at free-elem offset
    `start` (tiles are row-major in the free dims, so rows coalesce)."""
    a = t[:]
    return AP(a.tensor, a.offset + start, [list(a.ap[0]), [1, length]])


def _chain(nc, xt, v, c, o, w2):
    """The 5-op DVE chain over a [P, SB, w2] input tile.  All APs are flat
    [P, RB*w2] streams; the shifted taps write junk into each row's 2 pad
    columns, which the store skips.  v/c/o are [P, RB, w2] tiles."""
    L = RB * w2

    nc.vector.tensor_tensor(
        out=_flat(v, 0, L), in0=_flat(xt, 0, L), in1=_flat(xt, 2 * w2, L),
        op=MX,
    )
    nc.vector.tensor_tensor(
        out=_flat(c, 0, L), in0=_flat(v, 0, L), in1=_flat(xt, w2, L), op=MX
    )
    # m1 = max(c@col-1, c@col+1), in place (write trails both reads)
    nc.vector.tensor_tensor(
        out=_flat(c, 0, L - 2), in0=_flat(c, 0, L - 2), in1=_flat(c, 2, L - 2),
        op=MX,
    )
    # q = max(m1, v, 2*eps): the 2eps clamp makes the tail's compare imply
    # x > eps, so no final relu is needed.
    nc.vector.scalar_tensor_tensor(
        out=_flat(c, 0, L - 1), in0=_flat(c, 0, L - 1), scalar=EPS2,
        in1=_flat(v, 1, L - 1), op0=MX, op1=MX,
    )
    # out = select(q - eps < x, x - eps, 0), f16 on the write port
    nc.vector._custom_dve(
        NMS_TAIL, out=_flat(o, 0, L), in0=_flat(xt, w2 + 1, L),
        in1=_flat(c, 0, L), s0=EPS,
    )


def _emit_pipeline(nc, tc, x_d, o_d, out_row_stride, out_offset0, mode="full"):
    """Row-band pipeline; bands 0 and NBAND-1 are column-split into two
    half-width chains (hides the first load / last store)."""
    do_load = mode in ("full", "dmaonly", "loadonly")
    do_store = mode in ("full", "dmaonly", "storeonly")
    do_compute = mode in ("full", "nodma")

    def load(dst, t, cb, w2, strips=2):
        """Column-strip the load across `strips` dma queues so the transfer
        runs at strips x the single-queue rate (the consumer waits for all
        strips; only the critical first load needs strips=4)."""
        if do_load:
            bounds = [w2 * i // strips for i in range(strips + 1)]
            for s0, s1 in zip(bounds, bounds[1:]):
                nc.sync.dma_start(
                    out=dst[:, :, s0:s1],
                    in_=AP(
                        x_d.tensor,
                        t * BAND_H * WP2 + cb + s0,
                        [[RB * WP2, P], [WP2, SB], [1, s1 - s0]],
                    ),
                )
        elif do_compute:
            # nodma diagnostic: a tile must have >=1 writer for Tile's
            # allocator; a tiny corner memset keeps the bench un-bottlenecked
            nc.gpsimd.memset(dst[:, :, 0:2], 0.25)

    def store(src, t, cb, w, strips=2):
        if do_store:
            bounds = [w * i // strips for i in range(strips + 1)]
            for s0, s1 in zip(bounds, bounds[1:]):
                nc.sync.dma_start(
                    out=AP(
                        o_d.tensor,
                        out_offset0 + t * BAND_H * out_row_stride + cb + s0,
                        [[RB * out_row_stride, P], [out_row_stride, RB],
                         [1, s1 - s0]],
                    ),
                    in_=src[:, :, s0:s1],
                )

    with (
        tc.tile_pool(name="iox", bufs=2) as iox,
        tc.tile_pool(name="work", bufs=1) as wp,
        tc.tile_pool(name="ioo", bufs=2) as ioo,
    ):
        for t in range(NBAND):
            split = t in (0, NBAND - 1)
            if split and do_compute:
                for cb in (0, HALF):
                    first = t == 0 and cb == 0
                    last = t == NBAND - 1 and cb == HALF
                    xth = iox.tile([P, SB, HALF + 2], F32, tag="xt")
                    vh = wp.tile([P, RB, HALF + 2], F32, tag="v")
                    ch = wp.tile([P, RB, HALF + 2], F32, tag="c")
                    oh = ioo.tile([P, RB, HALF + 2], F16, tag="o")
                    load(xth, t, cb, HALF + 2, strips=4 if first else 2)
                    _chain(nc, xth, vh, ch, oh, HALF + 2)
                    store(oh, t, cb, HALF, strips=4 if last else 2)
                continue

            xt = iox.tile([P, SB, WP2], F32, tag="xt")
            o = ioo.tile([P, RB, WP2], F16, tag="o")
            load(xt, t, 0, WP2, strips=2)
            if do_compute:
                v = wp.tile([P, RB, WP2], F32, tag="v")
                c = wp.tile([P, RB, WP2], F32, tag="c")
                _chain(nc, xt, v, c, o, WP2)
            else:
                nc.gpsimd.memset(o[:, :, 0:2], 0.0)
            store(o, t, 0, W)


def _build_program():
    nc = bacc.Bacc(
        "TRN2",
        target_bir_lowering=False,
        debug=False,
        enable_asserts=False,
        num_devices=B,
    )
    x_d = nc.dram_tensor("x", [HP2, WP2], F32, kind="ExternalInput").ap()
    o_d = nc.dram_tensor("out", [H, W], F16, kind="ExternalOutput").ap()
    with tile.TileContext(nc) as tc:
        _emit_pipeline(nc, tc, x_d, o_d, W, 0)
    nc.compile()
    return nc


def _build_timing_program(niter=1, mode="full"):
    """Same pipeline repeated `niter` times by a device-side loop, writing
    out as f16 [HP2, WP2].  One execute performs niter full passes, so
    (wall(niter) - wall(1)) / (niter - 1) isolates device time from the
    (identical) transfer cost.  Border cells of out are never written."""
    nc = bacc.Bacc(
        "TRN2",
        target_bir_lowering=False,
        debug=False,
        enable_asserts=False,
        num_devices=B,
    )
    di = nc.dram_tensor("x", [1, 8], F32, kind="ExternalInput").ap()
    do = nc.dram_tensor("out", [1, 8], F32, kind="ExternalOutput").ap()
    # the working image lives in Internal DRAM scratch (contents irrelevant
    # for timing); external I/O is a tiny dummy so transfers are ~free.
    x_d = nc.dram_tensor("xi", [HP2, WP2], F32, kind="Internal").ap()
    o_d = nc.dram_tensor("oi", [HP2, WP2], F16, kind="Internal").ap()
    with tile.TileContext(nc) as tc:
        with tc.tile_pool(name="dummy", bufs=1) as dp:
            dt = dp.tile([1, 8], F32, tag="dummy")
            nc.sync.dma_start(out=dt[:], in_=di[:])
            nc.sync.dma_start(out=do[:], in_=dt[:])
        if niter == 1:
            _emit_pipeline(nc, tc, x_d, o_d, WP2, WP2 + 1, mode)
        else:
            with tc.For_i(0, niter, 1):
                _emit_pipeline(nc, tc, x_d, o_d, WP2, WP2 + 1, mode)
    nc.compile()
    return nc


_NC = None


def _get_program():
    global _NC
    if _NC is None:
        _NC = _build_program()
    return _NC


def kernel(x: np.ndarray) -> np.ndarray:
    x = np.asarray(x, dtype=np.float32)
    assert x.shape == (B, H, W), x.shape
    xpad = np.zeros((B, HP2, WP2), dtype=np.float32)
    xpad[:, 1 : H + 1, 1 : W + 1] = x
    nc = _get_program()
    in_maps = [{"x": xpad[i]} for i in range(B)]
    res = bass_utils.run_bass_kernel_spmd(nc, in_maps, core_ids=list(range(B)))
    return np.stack([r["out"] for r in res.results], axis=0).astype(np.float32)
